# revision 39
# baseline (speedup 1.0000x reference)
"""GATConv (multi-head graph attention) on 8 Trainium2 NeuronCores.

kernel(**inputs) takes the FULL numpy inputs and returns the FULL
[50000, 256] float32 output.  All floating-point math runs on-device in a
Bass/Tile kernel; the host only does index bookkeeping (edge sorting,
gather-index tables, 0/1 selector matrices) and shape padding.

Distribution: nodes are block-partitioned across the 8 cores.  Per core:
phase 0 projects its node slice (x @ [W | W@A] via fp32r matmuls) producing
h and the attention dot-products s; phase 1 computes softmax denominators
for its source nodes; phase 2 aggregates messages at its destination nodes.
Two AllGathers replicate the per-node tables between phases.

v2: dma_gather calls are batched per SUPER-block (amortizing the ~1us
SWDGE fixed overhead per call that dominated v1), tile counts are sized
per block position (max over cores) instead of a global max, and the
per-block sum-exp matmul is flipped (one-hot stationary, exp values
moving) so the result lands in [128 nodes, 4 heads] layout directly,
removing the PSUM transpose.

v3: the own-block S gathers (g2/gs) are gone: per-edge key-side s values
are selected on-chip instead, via sel = eqT^T @ s_block where eqT is the
PE-transposed streamed one-hot and s_block is a resident bf16 copy of
the core's own s table.  This halves the bytes drained through the
GPSIMD-serialized SWDGE gather pipe, which the v2 trace showed to be the
bottleneck (~3.4 ns/row serial, 90% GPSIMD busy).
"""

import sys

sys.path.insert(0, "/opt/trn_rl_repo")

import numpy as np

N_NODES = 50000
N_EDGES = 800000
IN_DIM = 512
HEADS = 4
OUT_DIM = 64
F = HEADS * OUT_DIM  # 256
N_CORES = 8
HALF = 32768  # int16 gather index range split
GROW = 384  # G table bf16 elems/row: 256 h | 8 (s_src f32) | 8 (recip f32) | pad
SROW = 64  # S table f32 elems/row: 4 s_src | 4 s_dst | pad  -> 256B
SUP1 = 6  # blocks per gather super-call, phase 1
SUP2 = 3  # blocks per gather super-call, phase 2


def _ceil_div(a, b):
    return (a + b - 1) // b


def _wrap16(arr_i16):
    """dma_gather idx layout: position i -> [i % 16, i // 16], x8 core groups."""
    n = arr_i16.shape[0]
    assert n % 16 == 0
    w = arr_i16.reshape(n // 16, 16).T
    return np.ascontiguousarray(np.tile(w, (8, 1)))


class PhaseMeta:
    """Host-built gather/selector metadata for one edge pass.

    Edges are keyed by `key` (block-local tiles; selector one-hot on
    key%128) and gather rows of the `other` node (split in lo/hi halves
    for int16 indexing).  Tiles are grouped into supers of `sup` blocks;
    within a super the layout is [all lo tiles by block | all hi tiles
    by block] so each super needs only 3 dma_gather calls (lo, hi, own).
    """

    def __init__(self, key, other, n_cores, nblk, sup):
        import ml_dtypes

        rpc = nblk * 128
        gblk = key >> 7
        core = gblk // nblk
        lb = gblk % nblk
        nsup = _ceil_div(nblk, sup)
        hcr = rpc // 2
        hi = ((other % rpc) >= hcr).astype(np.int64)
        sup_id = lb // sup

        order = np.lexsort((other, lb, hi, sup_id, core))
        core_s = core[order]
        lb_s = lb[order]
        hi_s = hi[order]
        key_s = key[order]
        other_s = other[order]

        cnt = np.zeros((n_cores, nblk, 2), np.int64)
        np.add.at(cnt, (core, lb, hi), 1)
        t_need = _ceil_div(cnt, 128).max(axis=0)  # [nblk, 2] max over cores
        self.t_lo = np.maximum(t_need[:, 0], 1)
        self.t_hi = np.maximum(t_need[:, 1], 1)

        self.nsup = nsup
        self.sup = sup
        self.st_lo = []  # lo tiles per super
        self.st_hi = []
        self.blocks = []  # per super: list of (lb, lo_t0, t_lo, hi_t0, t_hi)
        tile_start = np.zeros((nblk, 2), np.int64)
        sup_base = 0
        for s in range(nsup):
            mem = range(s * sup, min((s + 1) * sup, nblk))
            stl = int(self.t_lo[list(mem)].sum())
            sth = int(self.t_hi[list(mem)].sum())
            self.st_lo.append(stl)
            self.st_hi.append(sth)
            blks = []
            lo_o, hi_o = 0, stl
            for b in mem:
                tile_start[b, 0] = sup_base + lo_o
                tile_start[b, 1] = sup_base + hi_o
                blks.append((b, lo_o, int(self.t_lo[b]), hi_o, int(self.t_hi[b])))
                lo_o += int(self.t_lo[b])
                hi_o += int(self.t_hi[b])
            self.blocks.append(blks)
            sup_base += stl + sth
        self.total_tiles = sup_base
        self.st = [l + h for l, h in zip(self.st_lo, self.st_hi)]
        self.sup_off = np.cumsum([0] + self.st).tolist()

        # rank of each edge within its (core, lb, hi) group
        gid = (core_s * nblk + lb_s) * 2 + hi_s
        change = np.r_[True, gid[1:] != gid[:-1]]
        gstart = np.flatnonzero(change)
        grp = np.cumsum(change) - 1
        rank = np.arange(len(key)) - gstart[grp]
        pos = (core_s * self.total_tiles + tile_start[lb_s, hi_s]) * 128 + rank

        rows = n_cores * self.total_tiles * 128
        gidx = np.zeros(rows, np.int16)
        sidx = np.zeros(rows, np.int16)
        gidx[pos] = ((other_s // rpc) * hcr
                     + (other_s % rpc) - hcr * hi_s).astype(np.int16)
        sidx[pos] = (key_s - core_s * rpc).astype(np.int16)
        eq = np.zeros((rows, 128), ml_dtypes.bfloat16)
        eq[pos, key_s & 127] = 1.0

        per_core = self.total_tiles * 128
        self.gidx_c, self.sidx_c, self.eq_c = [], [], []
        for c in range(n_cores):
            sl = slice(c * per_core, (c + 1) * per_core)
            self.gidx_c.append(_wrap16(gidx[sl]))
            self.sidx_c.append(_wrap16(sidx[sl]))
            # [tiles*128 lanes, 128 rel] -> [128 lanes, tiles*128] so the
            # device-side load is a contiguous partition-major DMA (128 big
            # descriptors) instead of tiles*128 strided 256B descriptors.
            ec = eq[sl].reshape(self.total_tiles, 128, 128)
            ec = ec.transpose(1, 0, 2).reshape(128, self.total_tiles * 128)
            self.eq_c.append(np.ascontiguousarray(ec))


def _build_bass_program(npad, rpc, nblk, m1, m2, n_cores, enable_asserts=False):
    import concourse.bacc as bacc
    import concourse.mybir as mybir
    import concourse.tile as tile

    dt = mybir.dt
    Alu = mybir.AluOpType
    Act = mybir.ActivationFunctionType
    KC = IN_DIM // 128
    WCOL = F + 2 * HEADS  # 264
    H2 = 2 * HEADS
    f32r = dt.float32r
    bf16 = dt.bfloat16

    nc = bacc.Bacc(
        "TRN2",
        target_bir_lowering=False,
        debug=False,
        enable_asserts=enable_asserts,
        num_devices=n_cores,
        num_swdge_queues=4,
    )

    xT = nc.dram_tensor("xT", [IN_DIM, rpc], bf16, kind="ExternalInput")
    W_in = nc.dram_tensor("W", [IN_DIM, F], dt.float32, kind="ExternalInput")
    a_in = nc.dram_tensor("a", [HEADS, 2 * OUT_DIM], dt.float32, kind="ExternalInput")
    bias_in = nc.dram_tensor("bias", [1, F], dt.float32, kind="ExternalInput")
    p1_gidx = nc.dram_tensor("p1_gidx", [128, m1.total_tiles * 8], dt.int16, kind="ExternalInput")
    p1_eq = nc.dram_tensor("p1_eq", [128, m1.total_tiles * 128], bf16, kind="ExternalInput")
    p2_gidx = nc.dram_tensor("p2_gidx", [128, m2.total_tiles * 8], dt.int16, kind="ExternalInput")
    p2_eq = nc.dram_tensor("p2_eq", [128, m2.total_tiles * 128], bf16, kind="ExternalInput")
    out = nc.dram_tensor("out", [rpc, F], dt.float32, kind="ExternalOutput")

    with tile.TileContext(nc) as tc:
        with (
            tc.tile_pool(name="const", bufs=1) as cpool,
            tc.tile_pool(name="dram", bufs=1, space="DRAM") as dram,
        ):
            G_own = dram.tile([rpc, GROW], bf16)
            S_own = dram.tile([rpc, SROW], dt.float32)
            hcr = rpc // 2
            G_fullA = dram.tile([n_cores * hcr, GROW], bf16, addr_space="Shared")
            G_fullB = dram.tile([n_cores * hcr, GROW], bf16, addr_space="Shared")
            S_fullA = dram.tile([n_cores * hcr, SROW], dt.float32, addr_space="Shared")
            S_fullB = dram.tile([n_cores * hcr, SROW], dt.float32, addr_space="Shared")

            # ---------------- constants ----------------
            iota_i = cpool.tile([128, 128], dt.int32)
            nc.gpsimd.iota(iota_i[:], pattern=[[1, 128]], channel_multiplier=0)
            iota_f = cpool.tile([128, 128], dt.float32)
            nc.vector.tensor_copy(iota_f[:], iota_i[:])
            pidx_i = cpool.tile([128, 1], dt.int32)
            nc.gpsimd.iota(pidx_i[:], pattern=[[0, 1]], channel_multiplier=1)
            pidx_f = cpool.tile([128, 1], dt.float32)
            nc.vector.tensor_copy(pidx_f[:], pidx_i[:])
            ident = cpool.tile([128, 128], dt.float32)
            nc.vector.tensor_scalar(ident[:], iota_f[:], pidx_f[:], None, op0=Alu.is_equal)
            identb = cpool.tile([128, 128], bf16)
            nc.vector.tensor_copy(identb[:], ident[:])

            # resident bf16 copy of this core's s values: [s_src(4) s_dst(4)]
            s_res = cpool.tile([128, nblk * H2], bf16)

            bias_bc = cpool.tile([128, F], dt.float32)
            nc.sync.dma_start(bias_bc[:1, :], bias_in[:, :])
            nc.gpsimd.partition_broadcast(bias_bc[:], bias_bc[:1, :])

            A0 = cpool.tile([128, H2], dt.float32)
            A1 = cpool.tile([128, H2], dt.float32)
            nc.vector.memset(A0[:], 0.0)
            nc.vector.memset(A1[:], 0.0)
            for h in range(HEADS):
                dstA = A0 if h < 2 else A1
                p0 = (h % 2) * OUT_DIM
                nc.sync.dma_start(dstA[p0:p0 + OUT_DIM, h:h + 1], a_in[h:h + 1, 0:OUT_DIM])
                nc.sync.dma_start(
                    dstA[p0:p0 + OUT_DIM, HEADS + h:HEADS + h + 1],
                    a_in[h:h + 1, OUT_DIM:2 * OUT_DIM],
                )

            W_sb = cpool.tile([128, KC * WCOL], dt.float32)
            for kc in range(KC):
                nc.sync.dma_start(
                    W_sb[:, kc * WCOL:kc * WCOL + F], W_in[kc * 128:(kc + 1) * 128, :]
                )
            WT0 = cpool.tile([128, IN_DIM], dt.float32)
            WT1 = cpool.tile([128, IN_DIM], dt.float32)
            with tc.tile_pool(name="psum_pre", bufs=2, space="PSUM") as pp:
                for kc in range(KC):
                    for fc in range(2):
                        pt = pp.tile([128, 128], dt.float32, tag="tr")
                        nc.tensor.transpose(
                            pt[:],
                            W_sb[:, kc * WCOL + fc * 128:kc * WCOL + (fc + 1) * 128],
                            ident[:],
                        )
                        wt = WT0 if fc == 0 else WT1
                        nc.vector.tensor_copy(wt[:, kc * 128:(kc + 1) * 128], pt[:])
                for kc in range(KC):
                    pwa = pp.tile([128, H2], dt.float32, tag="wa")
                    for fc in range(2):
                        wt = WT0 if fc == 0 else WT1
                        A = A0 if fc == 0 else A1
                        nc.tensor.matmul(
                            pwa[:], wt[:, kc * 128:(kc + 1) * 128], A[:],
                            start=(fc == 0), stop=(fc == 1),
                        )
                    nc.vector.tensor_copy(W_sb[:, kc * WCOL + F:(kc + 1) * WCOL], pwa[:])
            W_sbr = cpool.tile([128, KC * WCOL], bf16)
            nc.vector.tensor_copy(W_sbr[:], W_sb[:])

            # ---------------- phase 0: projection ----------------
            with (
                tc.tile_pool(name="p0x", bufs=1) as p0x,
                tc.tile_pool(name="p0", bufs=3) as p0pool,
                tc.tile_pool(name="p0ps", bufs=2, space="PSUM") as p0ps,
            ):
                xres = p0x.tile([128, KC * rpc], bf16)
                for kc in range(KC):
                    nc.sync.dma_start(
                        xres[:, kc * rpc:(kc + 1) * rpc],
                        xT[kc * 128:(kc + 1) * 128, :],
                    )
                xres3 = xres[:].rearrange("p (k n) -> p k n", k=KC)
                for r in range(nblk):
                    xtr = p0pool.tile([128, KC * 128], bf16, tag="xtr")
                    nc.vector.tensor_copy(
                        xtr[:].rearrange("p (k n) -> p k n", k=KC),
                        xres3[:, :, r * 128:(r + 1) * 128],
                    )
                    ps = p0ps.tile([128, WCOL], dt.float32, tag="hps")
                    for kc in range(KC):
                        nc.tensor.matmul(
                            ps[:], xtr[:, kc * 128:(kc + 1) * 128],
                            W_sbr[:, kc * WCOL:(kc + 1) * WCOL],
                            start=(kc == 0), stop=(kc == KC - 1),
                        )
                    gsb = p0pool.tile([128, GROW], bf16, tag="gsb")
                    nc.vector.memset(gsb[:, F + 2 * H2:GROW], 0.0)
                    nc.vector.tensor_copy(gsb[:, 0:F], ps[:, 0:F])  # h -> bf16
                    nc.vector.tensor_copy(  # s_src raw f32 bits
                        gsb[:, F:F + H2].bitcast(dt.float32), ps[:, F:F + HEADS]
                    )
                    nc.vector.memset(gsb[:, F + H2:F + 2 * H2], 0.0)  # recip slot
                    nc.sync.dma_start(G_own[r * 128:(r + 1) * 128, :], gsb[:])
                    ssb = p0pool.tile([128, SROW], dt.float32, tag="ssb")
                    nc.vector.memset(ssb[:, H2:SROW], 0.0)
                    nc.vector.tensor_copy(ssb[:, 0:H2], ps[:, F:WCOL])
                    nc.vector.tensor_copy(
                        s_res[:, r * H2:(r + 1) * H2], ps[:, F:WCOL]
                    )
                    nc.sync.dma_start(S_own[r * 128:(r + 1) * 128, :], ssb[:])

            nc.gpsimd.collective_compute(
                "AllGather", Alu.bypass,
                ins=[S_own[0:hcr, :].opt()], outs=[S_fullA[:].opt()],
                replica_groups=[list(range(n_cores))],
            )
            # serialize the chunk pair: B must not be in flight with A
            # (concurrent collectives raced; chain B's input on A's output)
            dmy_s = cpool.tile([1, 8], dt.float32)
            nc.sync.dma_start(dmy_s[:], S_fullA[0:1, 0:8])
            nc.sync.dma_start(S_own[rpc - 1:rpc, SROW - 8:SROW], dmy_s[:])
            nc.gpsimd.collective_compute(
                "AllGather", Alu.bypass,
                ins=[S_own[hcr:rpc, :].opt()], outs=[S_fullB[:].opt()],
                replica_groups=[list(range(n_cores))],
            )

            # ---------------- phase 1: softmax denominators ----------------
            with (
                tc.tile_pool(name="p1", bufs=3) as p1pool,
                tc.tile_pool(name="p1ps", bufs=2, space="PSUM") as p1ps,
            ):
                for s in range(m1.nsup):
                    ST = m1.st[s]
                    STl = m1.st_lo[s]
                    STh = m1.st_hi[s]
                    toff = m1.sup_off[s]
                    co = toff * 8
                    gix = p1pool.tile([128, ST * 8], dt.int16, tag="gix")
                    nc.sync.dma_start(gix[:], p1_gidx[:, co:co + ST * 8])

                    g1 = p1pool.tile([128, ST, SROW], dt.float32, tag="g1")
                    La = STl // 2
                    Ha = STh // 2
                    calls = [
                        (0, La, S_fullA[:, :]),
                        (La, STl - La, S_fullA[:, :]),
                        (STl, Ha, S_fullB[:, :]),
                        (STl + Ha, STh - Ha, S_fullB[:, :]),
                    ]
                    for i, (t0, nt, src_ap) in enumerate(calls):
                        if nt == 0:
                            continue
                        nc.gpsimd.dma_gather(
                            g1[:, t0:t0 + nt, :], src_ap,
                            gix[:, t0 * 8:(t0 + nt) * 8], nt * 128, nt * 128,
                            SROW, single_packet=False, queue_num=i,
                        )
                    eq_sb = p1pool.tile([128, ST, 128], bf16, tag="eqs1")
                    nc.sync.dma_start(
                        eq_sb[:],
                        p1_eq[:, toff * 128:(toff + ST) * 128].rearrange(
                            "l (t r) -> l t r", r=128
                        ),
                    )

                    # per-edge key-side s_src via on-chip select:
                    # eqT = transpose(eq); s_sel[lane] = eqT^T @ s_res[block]
                    tile_block = [0] * ST
                    for (b, lo0, tl, hi0, th) in m1.blocks[s]:
                        for t in range(lo0, lo0 + tl):
                            tile_block[t] = b
                        for t in range(hi0, hi0 + th):
                            tile_block[t] = b
                    zp = p1ps.tile([128, ST * HEADS], dt.float32, tag="zp")
                    ptr = eqt = None
                    for t in range(ST):
                        k = t % 4
                        if k == 0:
                            ptr = p1ps.tile([128, 512], bf16, tag="ptr")
                            eqt = p1pool.tile([128, 512], bf16, tag="eqt")
                        nc.tensor.transpose(
                            ptr[:, k * 128:(k + 1) * 128], eq_sb[:, t, :],
                            identb[:],
                        )
                        if k == 3 or t == ST - 1:
                            if (t // 4) % 2 == 0:
                                nc.scalar.copy(eqt[:, :(k + 1) * 128],
                                               ptr[:, :(k + 1) * 128])
                            else:
                                nc.vector.tensor_copy(eqt[:, :(k + 1) * 128],
                                                      ptr[:, :(k + 1) * 128])
                            for tt in range(t - k, t + 1):
                                kk = tt % 4
                                b = tile_block[tt]
                                nc.tensor.matmul(
                                    zp[:, tt * HEADS:(tt + 1) * HEADS],
                                    eqt[:, kk * 128:(kk + 1) * 128],
                                    s_res[:, b * H2:b * H2 + HEADS],
                                    start=True, stop=True,
                                )

                    # v = exp(leaky_relu(s_src + s_dst)) in bf16
                    z = p1pool.tile([128, ST * HEADS], dt.float32, tag="z1")
                    z3 = z[:].rearrange("p (t h) -> p t h", h=HEADS)
                    nc.vector.tensor_tensor(
                        z3, zp[:].rearrange("p (t h) -> p t h", h=HEADS),
                        g1[:, :, HEADS:H2], op=Alu.add
                    )
                    nc.vector.scalar_tensor_tensor(
                        z[:], z[:], 0.2, z[:], op0=Alu.mult, op1=Alu.max
                    )
                    v = p1pool.tile([128, ST * HEADS], bf16, tag="v1")
                    nc.scalar.activation(v[:], z[:], Act.Exp)

                    # per block: sumexp[128 nodes, 4] += eq_t^T @ v_t
                    for (b, lo0, tl, hi0, th) in m1.blocks[s]:
                        ps1 = p1ps.tile([128, HEADS], dt.float32, tag="ps1")
                        tiles = list(range(lo0, lo0 + tl)) + list(range(hi0, hi0 + th))
                        for i, t in enumerate(tiles):
                            nc.tensor.matmul(
                                ps1[:], eq_sb[:, t, :],
                                v[:, t * HEADS:(t + 1) * HEADS],
                                start=(i == 0), stop=(i == len(tiles) - 1),
                            )
                        se = p1pool.tile([128, HEADS], dt.float32, tag="se")
                        nc.vector.tensor_scalar_add(se[:], ps1[:], 1e-10)
                        recd = p1pool.tile([128, HEADS], dt.float32, tag="recd")
                        nc.vector.reciprocal(recd[:], se[:])
                        nc.sync.dma_start(
                            G_own[b * 128:(b + 1) * 128,
                                  F + H2:F + 2 * H2].bitcast(dt.float32),
                            recd[:],
                        )

            nc.gpsimd.collective_compute(
                "AllGather", Alu.bypass,
                ins=[G_own[0:hcr, :].opt()], outs=[G_fullA[:].opt()],
                replica_groups=[list(range(n_cores))],
            )
            dmy_g = cpool.tile([1, 8], bf16)
            nc.sync.dma_start(dmy_g[:], G_fullA[0:1, 0:8])
            nc.sync.dma_start(G_own[rpc - 1:rpc, GROW - 8:GROW], dmy_g[:])
            nc.gpsimd.collective_compute(
                "AllGather", Alu.bypass,
                ins=[G_own[hcr:rpc, :].opt()], outs=[G_fullB[:].opt()],
                replica_groups=[list(range(n_cores))],
            )

            # ---------------- phase 2: aggregate messages ----------------
            with (
                tc.tile_pool(name="p2", bufs=3) as p2pool,
                tc.tile_pool(name="p2m", bufs=3) as p2m,
                tc.tile_pool(name="p2ps", bufs=2, space="PSUM") as p2ps,
            ):
                for s in range(m2.nsup):
                    ST = m2.st[s]
                    STl = m2.st_lo[s]
                    STh = m2.st_hi[s]
                    toff = m2.sup_off[s]
                    co = toff * 8
                    gix = p2pool.tile([128, ST * 8], dt.int16, tag="gix2")
                    nc.sync.dma_start(gix[:], p2_gidx[:, co:co + ST * 8])

                    g = p2pool.tile([128, ST, GROW], bf16, tag="g")
                    La = STl // 2
                    Ha = STh // 2
                    calls = [
                        (0, La, G_fullA[:, :]),
                        (La, STl - La, G_fullA[:, :]),
                        (STl, Ha, G_fullB[:, :]),
                        (STl + Ha, STh - Ha, G_fullB[:, :]),
                    ]
                    for i, (t0, nt, src_ap) in enumerate(calls):
                        if nt == 0:
                            continue
                        nc.gpsimd.dma_gather(
                            g[:, t0:t0 + nt, :], src_ap,
                            gix[:, t0 * 8:(t0 + nt) * 8], nt * 128, nt * 128,
                            GROW, single_packet=False, queue_num=i,
                        )
                    eq_sb = p2pool.tile([128, ST, 128], bf16, tag="eqs2")
                    nc.sync.dma_start(
                        eq_sb[:],
                        p2_eq[:, toff * 128:(toff + ST) * 128].rearrange(
                            "l (t r) -> l t r", r=128
                        ),
                    )

                    # per-edge key-side s_dst via on-chip select (as phase 1)
                    tile_block = [0] * ST
                    for (b, lo0, tl, hi0, th) in m2.blocks[s]:
                        for t in range(lo0, lo0 + tl):
                            tile_block[t] = b
                        for t in range(hi0, hi0 + th):
                            tile_block[t] = b
                    zp = p2ps.tile([128, ST * HEADS], dt.float32, tag="zp2")
                    ptr = eqt = None
                    for t in range(ST):
                        k = t % 4
                        if k == 0:
                            ptr = p2ps.tile([128, 512], bf16, tag="ptr2")
                            eqt = p2pool.tile([128, 512], bf16, tag="eqt2")
                        nc.tensor.transpose(
                            ptr[:, k * 128:(k + 1) * 128], eq_sb[:, t, :],
                            identb[:],
                        )
                        if k == 3 or t == ST - 1:
                            if (t // 4) % 2 == 0:
                                nc.scalar.copy(eqt[:, :(k + 1) * 128],
                                               ptr[:, :(k + 1) * 128])
                            else:
                                nc.vector.tensor_copy(eqt[:, :(k + 1) * 128],
                                                      ptr[:, :(k + 1) * 128])
                            for tt in range(t - k, t + 1):
                                kk = tt % 4
                                b = tile_block[tt]
                                nc.tensor.matmul(
                                    zp[:, tt * HEADS:(tt + 1) * HEADS],
                                    eqt[:, kk * 128:(kk + 1) * 128],
                                    s_res[:, b * H2 + HEADS:(b + 1) * H2],
                                    start=True, stop=True,
                                )

                    # alpha = exp(leaky_relu(s_src + s_dst)) * recip  (f32)
                    al = p2pool.tile([128, ST * HEADS], dt.float32, tag="al")
                    al3 = al[:].rearrange("p (t h) -> p t h", h=HEADS)
                    nc.vector.tensor_tensor(
                        al3,
                        g[:, :, F:F + H2].bitcast(dt.float32),
                        zp[:].rearrange("p (t h) -> p t h", h=HEADS),
                        op=Alu.add,
                    )
                    nc.vector.scalar_tensor_tensor(
                        al[:], al[:], 0.2, al[:], op0=Alu.mult, op1=Alu.max
                    )
                    nc.scalar.activation(al[:], al[:], Act.Exp)
                    nc.vector.tensor_tensor(
                        al3, al3,
                        g[:, :, F + H2:F + 2 * H2].bitcast(dt.float32),
                        op=Alu.mult,
                    )

                    for (b, lo0, tl, hi0, th) in m2.blocks[s]:
                        ps2 = p2ps.tile([128, F], dt.float32, tag="ps2")
                        tiles = list(range(lo0, lo0 + tl)) + list(range(hi0, hi0 + th))
                        for i, t in enumerate(tiles):
                            alpha_b = al[:, t * HEADS:(t + 1) * HEADS].unsqueeze(
                                2
                            ).broadcast_to([128, HEADS, OUT_DIM])
                            msg = p2m.tile([128, F], bf16, tag="msg")
                            nc.vector.tensor_tensor(
                                msg[:].rearrange("p (h d) -> p h d", h=HEADS),
                                g[:, t, 0:F].rearrange("p (h d) -> p h d", h=HEADS),
                                alpha_b,
                                op=Alu.mult,
                            )
                            nc.tensor.matmul(
                                ps2[:], eq_sb[:, t, :], msg[:],
                                start=(i == 0), stop=(i == len(tiles) - 1),
                            )
                        osb = p2m.tile([128, F], dt.float32, tag="osb")
                        nc.vector.tensor_tensor(osb[:], ps2[:], bias_bc[:], op=Alu.add)
                        nc.sync.dma_start(out[b * 128:(b + 1) * 128, :], osb[:])

    nc.compile()
    return nc


def _gat_forward(x, edges, W, a, bias, n_nodes, n_cores, run_opts=None):
    npad = _ceil_div(n_nodes, n_cores * 128) * n_cores * 128
    rpc = npad // n_cores
    nblk = rpc // 128

    src = edges[:, 0].astype(np.int64)
    dst = edges[:, 1].astype(np.int64)
    m1 = PhaseMeta(src, dst, n_cores, nblk, SUP1)
    m2 = PhaseMeta(dst, src, n_cores, nblk, SUP2)

    nc = _build_bass_program(npad, rpc, nblk, m1, m2, n_cores)

    import ml_dtypes

    x_pad = np.zeros((npad, IN_DIM), np.float32)
    x_pad[:n_nodes] = x
    xT = np.ascontiguousarray(x_pad.T.astype(ml_dtypes.bfloat16))

    in_maps = []
    for c in range(n_cores):
        in_maps.append({
            "xT": np.ascontiguousarray(xT[:, c * rpc:(c + 1) * rpc]),
            "W": np.ascontiguousarray(W.astype(np.float32)),
            "a": np.ascontiguousarray(a.astype(np.float32)),
            "bias": np.ascontiguousarray(bias.astype(np.float32).reshape(1, F)),
            "p1_gidx": m1.gidx_c[c], "p1_eq": m1.eq_c[c],
            "p2_gidx": m2.gidx_c[c], "p2_eq": m2.eq_c[c],
        })

    from concourse.bass_utils import run_bass_kernel_spmd

    res = run_bass_kernel_spmd(
        nc, in_maps, core_ids=list(range(n_cores)), **(run_opts or {})
    )
    out = np.concatenate([r["out"] for r in res.results], axis=0)
    return out[:n_nodes], res


def kernel(x, edges, W, a, bias):
    x = np.asarray(x, np.float32)
    edges = np.asarray(edges)
    W = np.asarray(W, np.float32)
    a = np.asarray(a, np.float32)
    bias = np.asarray(bias, np.float32)
    out, _ = _gat_forward(x, edges, W, a, bias, N_NODES, N_CORES)
    return out


# revision 40
# speedup vs baseline: 1.0656x; 1.0656x over previous
"""GATConv (multi-head graph attention) on 8 Trainium2 NeuronCores.

kernel(**inputs) takes the FULL numpy inputs and returns the FULL
[50000, 256] float32 output.  All floating-point math runs on-device in a
Bass/Tile kernel; the host only does index bookkeeping (edge sorting,
gather-index tables, 0/1 selector matrices) and shape padding.

Distribution: nodes are block-partitioned across the 8 cores.  Per core:
phase 0 projects its node slice (x @ [W | W@A] via fp32r matmuls) producing
h and the attention dot-products s; phase 1 computes softmax denominators
for its source nodes; phase 2 aggregates messages at its destination nodes.
Two AllGathers replicate the per-node tables between phases.

v2: dma_gather calls are batched per SUPER-block (amortizing the ~1us
SWDGE fixed overhead per call that dominated v1), tile counts are sized
per block position (max over cores) instead of a global max, and the
per-block sum-exp matmul is flipped (one-hot stationary, exp values
moving) so the result lands in [128 nodes, 4 heads] layout directly,
removing the PSUM transpose.

v3: the own-block S gathers (g2/gs) are gone: per-edge key-side s values
are selected on-chip instead, via sel = eqT^T @ s_block where eqT is the
PE-transposed streamed one-hot and s_block is a resident bf16 copy of
the core's own s table.  This halves the bytes drained through the
GPSIMD-serialized SWDGE gather pipe, which the v2 trace showed to be the
bottleneck (~3.4 ns/row serial, 90% GPSIMD busy).
"""

import sys

sys.path.insert(0, "/opt/trn_rl_repo")

import numpy as np

N_NODES = 50000
N_EDGES = 800000
IN_DIM = 512
HEADS = 4
OUT_DIM = 64
F = HEADS * OUT_DIM  # 256
N_CORES = 8
HALF = 32768  # int16 gather index range split
GROW = 384  # G table bf16 elems/row: 256 h | 8 (s_src f32) | 8 (recip f32) | pad
SROW = 64  # S table f32 elems/row: 4 s_src | 4 s_dst | pad  -> 256B
SUP1 = 6  # blocks per gather super-call, phase 1
SUP2 = 3  # blocks per gather super-call, phase 2


def _ceil_div(a, b):
    return (a + b - 1) // b


def _wrap16(arr_i16):
    """dma_gather idx layout: position i -> [i % 16, i // 16], x8 core groups."""
    n = arr_i16.shape[0]
    assert n % 16 == 0
    w = arr_i16.reshape(n // 16, 16).T
    return np.ascontiguousarray(np.tile(w, (8, 1)))


class PhaseMeta:
    """Host-built gather/selector metadata for one edge pass.

    Edges are keyed by `key` (block-local tiles; selector one-hot on
    key%128) and gather rows of the `other` node (split in lo/hi halves
    for int16 indexing).  Tiles are grouped into supers of `sup` blocks;
    within a super the layout is [all lo tiles by block | all hi tiles
    by block] so each super needs only 3 dma_gather calls (lo, hi, own).
    """

    def __init__(self, key, other, n_cores, nblk, sup):
        import ml_dtypes

        rpc = nblk * 128
        gblk = key >> 7
        core = gblk // nblk
        lb = gblk % nblk
        nsup = _ceil_div(nblk, sup)
        hcr = rpc // 2
        hi = ((other % rpc) >= hcr).astype(np.int64)
        sup_id = lb // sup

        order = np.lexsort((other, lb, hi, sup_id, core))
        core_s = core[order]
        lb_s = lb[order]
        hi_s = hi[order]
        key_s = key[order]
        other_s = other[order]

        cnt = np.zeros((n_cores, nblk, 2), np.int64)
        np.add.at(cnt, (core, lb, hi), 1)
        t_need = _ceil_div(cnt, 128).max(axis=0)  # [nblk, 2] max over cores
        self.t_lo = np.maximum(t_need[:, 0], 1)
        self.t_hi = np.maximum(t_need[:, 1], 1)

        self.nsup = nsup
        self.sup = sup
        self.st_lo = []  # lo tiles per super
        self.st_hi = []
        self.blocks = []  # per super: list of (lb, lo_t0, t_lo, hi_t0, t_hi)
        tile_start = np.zeros((nblk, 2), np.int64)
        sup_base = 0
        for s in range(nsup):
            mem = range(s * sup, min((s + 1) * sup, nblk))
            stl = int(self.t_lo[list(mem)].sum())
            sth = int(self.t_hi[list(mem)].sum())
            self.st_lo.append(stl)
            self.st_hi.append(sth)
            blks = []
            lo_o, hi_o = 0, stl
            for b in mem:
                tile_start[b, 0] = sup_base + lo_o
                tile_start[b, 1] = sup_base + hi_o
                blks.append((b, lo_o, int(self.t_lo[b]), hi_o, int(self.t_hi[b])))
                lo_o += int(self.t_lo[b])
                hi_o += int(self.t_hi[b])
            self.blocks.append(blks)
            sup_base += stl + sth
        self.total_tiles = sup_base
        self.st = [l + h for l, h in zip(self.st_lo, self.st_hi)]
        self.sup_off = np.cumsum([0] + self.st).tolist()

        # rank of each edge within its (core, lb, hi) group
        gid = (core_s * nblk + lb_s) * 2 + hi_s
        change = np.r_[True, gid[1:] != gid[:-1]]
        gstart = np.flatnonzero(change)
        grp = np.cumsum(change) - 1
        rank = np.arange(len(key)) - gstart[grp]
        pos = (core_s * self.total_tiles + tile_start[lb_s, hi_s]) * 128 + rank

        rows = n_cores * self.total_tiles * 128
        gidx = np.zeros(rows, np.int16)
        sidx = np.zeros(rows, np.int16)
        gidx[pos] = ((other_s // rpc) * hcr
                     + (other_s % rpc) - hcr * hi_s).astype(np.int16)
        sidx[pos] = (key_s - core_s * rpc).astype(np.int16)
        eq = np.zeros((rows, 128), ml_dtypes.bfloat16)
        eq[pos, key_s & 127] = 1.0

        per_core = self.total_tiles * 128
        self.gidx_c, self.sidx_c, self.eq_c = [], [], []
        for c in range(n_cores):
            sl = slice(c * per_core, (c + 1) * per_core)
            self.gidx_c.append(_wrap16(gidx[sl]))
            self.sidx_c.append(_wrap16(sidx[sl]))
            # [tiles*128 lanes, 128 rel] -> [128 lanes, tiles*128] so the
            # device-side load is a contiguous partition-major DMA (128 big
            # descriptors) instead of tiles*128 strided 256B descriptors.
            ec = eq[sl].reshape(self.total_tiles, 128, 128)
            ec = ec.transpose(1, 0, 2).reshape(128, self.total_tiles * 128)
            self.eq_c.append(np.ascontiguousarray(ec))


def _build_bass_program(npad, rpc, nblk, m1, m2, n_cores, enable_asserts=False):
    import concourse.bacc as bacc
    import concourse.mybir as mybir
    import concourse.tile as tile

    dt = mybir.dt
    Alu = mybir.AluOpType
    Act = mybir.ActivationFunctionType
    KC = IN_DIM // 128
    WCOL = F + 2 * HEADS  # 264
    H2 = 2 * HEADS
    f32r = dt.float32r
    bf16 = dt.bfloat16

    nc = bacc.Bacc(
        "TRN2",
        target_bir_lowering=False,
        debug=False,
        enable_asserts=enable_asserts,
        num_devices=n_cores,
        num_swdge_queues=4,
    )

    xT = nc.dram_tensor("xT", [IN_DIM, rpc], bf16, kind="ExternalInput")
    W_in = nc.dram_tensor("W", [IN_DIM, F], dt.float32, kind="ExternalInput")
    a_in = nc.dram_tensor("a", [HEADS, 2 * OUT_DIM], dt.float32, kind="ExternalInput")
    bias_in = nc.dram_tensor("bias", [1, F], dt.float32, kind="ExternalInput")
    p1_gidx = nc.dram_tensor("p1_gidx", [128, m1.total_tiles * 8], dt.int16, kind="ExternalInput")
    p1_eq = nc.dram_tensor("p1_eq", [128, m1.total_tiles * 128], bf16, kind="ExternalInput")
    p2_gidx = nc.dram_tensor("p2_gidx", [128, m2.total_tiles * 8], dt.int16, kind="ExternalInput")
    p2_eq = nc.dram_tensor("p2_eq", [128, m2.total_tiles * 128], bf16, kind="ExternalInput")
    out = nc.dram_tensor("out", [rpc, F], dt.float32, kind="ExternalOutput")

    with tile.TileContext(nc) as tc:
        with (
            tc.tile_pool(name="const", bufs=1) as cpool,
            tc.tile_pool(name="dram", bufs=1, space="DRAM") as dram,
        ):
            hcr = rpc // 2
            G_ownA = dram.tile([hcr, GROW], bf16)
            G_ownB = dram.tile([rpc - hcr, GROW], bf16)
            S_ownA = dram.tile([hcr, SROW], dt.float32)
            S_ownB = dram.tile([rpc - hcr, SROW], dt.float32)
            G_fullA = dram.tile([n_cores * hcr, GROW], bf16, addr_space="Shared")
            G_fullB = dram.tile([n_cores * hcr, GROW], bf16, addr_space="Shared")
            S_fullA = dram.tile([n_cores * hcr, SROW], dt.float32, addr_space="Shared")
            S_fullB = dram.tile([n_cores * hcr, SROW], dt.float32, addr_space="Shared")

            # ---------------- constants ----------------
            iota_i = cpool.tile([128, 128], dt.int32)
            nc.gpsimd.iota(iota_i[:], pattern=[[1, 128]], channel_multiplier=0)
            iota_f = cpool.tile([128, 128], dt.float32)
            nc.vector.tensor_copy(iota_f[:], iota_i[:])
            pidx_i = cpool.tile([128, 1], dt.int32)
            nc.gpsimd.iota(pidx_i[:], pattern=[[0, 1]], channel_multiplier=1)
            pidx_f = cpool.tile([128, 1], dt.float32)
            nc.vector.tensor_copy(pidx_f[:], pidx_i[:])
            ident = cpool.tile([128, 128], dt.float32)
            nc.vector.tensor_scalar(ident[:], iota_f[:], pidx_f[:], None, op0=Alu.is_equal)
            identb = cpool.tile([128, 128], bf16)
            nc.vector.tensor_copy(identb[:], ident[:])

            # resident bf16 copy of this core's s values: [s_src(4) s_dst(4)]
            s_res = cpool.tile([128, nblk * H2], bf16)

            bias_bc = cpool.tile([128, F], dt.float32)
            nc.sync.dma_start(bias_bc[:1, :], bias_in[:, :])
            nc.gpsimd.partition_broadcast(bias_bc[:], bias_bc[:1, :])

            A0 = cpool.tile([128, H2], dt.float32)
            A1 = cpool.tile([128, H2], dt.float32)
            nc.vector.memset(A0[:], 0.0)
            nc.vector.memset(A1[:], 0.0)
            for h in range(HEADS):
                dstA = A0 if h < 2 else A1
                p0 = (h % 2) * OUT_DIM
                nc.sync.dma_start(dstA[p0:p0 + OUT_DIM, h:h + 1], a_in[h:h + 1, 0:OUT_DIM])
                nc.sync.dma_start(
                    dstA[p0:p0 + OUT_DIM, HEADS + h:HEADS + h + 1],
                    a_in[h:h + 1, OUT_DIM:2 * OUT_DIM],
                )

            W_sb = cpool.tile([128, KC * WCOL], dt.float32)
            for kc in range(KC):
                nc.sync.dma_start(
                    W_sb[:, kc * WCOL:kc * WCOL + F], W_in[kc * 128:(kc + 1) * 128, :]
                )
            WT0 = cpool.tile([128, IN_DIM], dt.float32)
            WT1 = cpool.tile([128, IN_DIM], dt.float32)
            with tc.tile_pool(name="psum_pre", bufs=2, space="PSUM") as pp:
                for kc in range(KC):
                    for fc in range(2):
                        pt = pp.tile([128, 128], dt.float32, tag="tr")
                        nc.tensor.transpose(
                            pt[:],
                            W_sb[:, kc * WCOL + fc * 128:kc * WCOL + (fc + 1) * 128],
                            ident[:],
                        )
                        wt = WT0 if fc == 0 else WT1
                        nc.vector.tensor_copy(wt[:, kc * 128:(kc + 1) * 128], pt[:])
                for kc in range(KC):
                    pwa = pp.tile([128, H2], dt.float32, tag="wa")
                    for fc in range(2):
                        wt = WT0 if fc == 0 else WT1
                        A = A0 if fc == 0 else A1
                        nc.tensor.matmul(
                            pwa[:], wt[:, kc * 128:(kc + 1) * 128], A[:],
                            start=(fc == 0), stop=(fc == 1),
                        )
                    nc.vector.tensor_copy(W_sb[:, kc * WCOL + F:(kc + 1) * WCOL], pwa[:])
            W_sbr = cpool.tile([128, KC * WCOL], bf16)
            nc.vector.tensor_copy(W_sbr[:], W_sb[:])

            # ---------------- phase 0: projection ----------------
            with (
                tc.tile_pool(name="p0x", bufs=1) as p0x,
                tc.tile_pool(name="p0", bufs=3) as p0pool,
                tc.tile_pool(name="p0ps", bufs=2, space="PSUM") as p0ps,
            ):
                xres = p0x.tile([128, KC * rpc], bf16)
                for kc in range(KC):
                    nc.sync.dma_start(
                        xres[:, kc * rpc:(kc + 1) * rpc],
                        xT[kc * 128:(kc + 1) * 128, :],
                    )
                xres3 = xres[:].rearrange("p (k n) -> p k n", k=KC)
                for r in range(nblk):
                    xtr = p0pool.tile([128, KC * 128], bf16, tag="xtr")
                    nc.vector.tensor_copy(
                        xtr[:].rearrange("p (k n) -> p k n", k=KC),
                        xres3[:, :, r * 128:(r + 1) * 128],
                    )
                    ps = p0ps.tile([128, WCOL], dt.float32, tag="hps")
                    for kc in range(KC):
                        nc.tensor.matmul(
                            ps[:], xtr[:, kc * 128:(kc + 1) * 128],
                            W_sbr[:, kc * WCOL:(kc + 1) * WCOL],
                            start=(kc == 0), stop=(kc == KC - 1),
                        )
                    gsb = p0pool.tile([128, GROW], bf16, tag="gsb")
                    nc.vector.memset(gsb[:, F + 2 * H2:GROW], 0.0)
                    nc.vector.tensor_copy(gsb[:, 0:F], ps[:, 0:F])  # h -> bf16
                    nc.vector.tensor_copy(  # s_src raw f32 bits
                        gsb[:, F:F + H2].bitcast(dt.float32), ps[:, F:F + HEADS]
                    )
                    nc.vector.memset(gsb[:, F + H2:F + 2 * H2], 0.0)  # recip slot
                    r0, r1 = r * 128, (r + 1) * 128
                    if r1 <= hcr:
                        nc.sync.dma_start(G_ownA[r0:r1, :], gsb[:])
                    elif r0 >= hcr:
                        nc.sync.dma_start(G_ownB[r0 - hcr:r1 - hcr, :], gsb[:])
                    else:
                        cut = hcr - r0
                        nc.sync.dma_start(G_ownA[r0:hcr, :], gsb[0:cut, :])
                        nc.sync.dma_start(G_ownB[0:r1 - hcr, :], gsb[cut:128, :])
                    ssb = p0pool.tile([128, SROW], dt.float32, tag="ssb")
                    nc.vector.memset(ssb[:, H2:SROW], 0.0)
                    nc.vector.tensor_copy(ssb[:, 0:H2], ps[:, F:WCOL])
                    nc.vector.tensor_copy(
                        s_res[:, r * H2:(r + 1) * H2], ps[:, F:WCOL]
                    )
                    if r1 <= hcr:
                        nc.sync.dma_start(S_ownA[r0:r1, :], ssb[:])
                    elif r0 >= hcr:
                        nc.sync.dma_start(S_ownB[r0 - hcr:r1 - hcr, :], ssb[:])
                    else:
                        cut = hcr - r0
                        nc.sync.dma_start(S_ownA[r0:hcr, :], ssb[0:cut, :])
                        nc.sync.dma_start(S_ownB[0:r1 - hcr, :], ssb[cut:128, :])

            nc.gpsimd.collective_compute(
                "AllGather", Alu.bypass,
                ins=[S_ownA[:].opt()], outs=[S_fullA[:].opt()],
                replica_groups=[list(range(n_cores))],
            )
            # serialize the chunk pair: B must not be in flight with A
            # (concurrent collectives raced; chain B's input on A's output)
            dmy_s = cpool.tile([1, 8], dt.float32)
            nc.sync.dma_start(dmy_s[:], S_fullA[0:1, 0:8])
            nc.sync.dma_start(S_ownB[rpc - hcr - 1:rpc - hcr, SROW - 8:SROW], dmy_s[:])
            nc.gpsimd.collective_compute(
                "AllGather", Alu.bypass,
                ins=[S_ownB[:].opt()], outs=[S_fullB[:].opt()],
                replica_groups=[list(range(n_cores))],
            )

            # ---------------- phase 1: softmax denominators ----------------
            with (
                tc.tile_pool(name="p1", bufs=3) as p1pool,
                tc.tile_pool(name="p1ps", bufs=2, space="PSUM") as p1ps,
            ):
                for s in range(m1.nsup):
                    ST = m1.st[s]
                    STl = m1.st_lo[s]
                    STh = m1.st_hi[s]
                    toff = m1.sup_off[s]
                    co = toff * 8
                    gix = p1pool.tile([128, ST * 8], dt.int16, tag="gix")
                    nc.sync.dma_start(gix[:], p1_gidx[:, co:co + ST * 8])

                    g1 = p1pool.tile([128, ST, SROW], dt.float32, tag="g1")
                    La = STl // 2
                    Ha = STh // 2
                    calls = [
                        (0, La, S_fullA[:, :]),
                        (La, STl - La, S_fullA[:, :]),
                        (STl, Ha, S_fullB[:, :]),
                        (STl + Ha, STh - Ha, S_fullB[:, :]),
                    ]
                    for i, (t0, nt, src_ap) in enumerate(calls):
                        if nt == 0:
                            continue
                        nc.gpsimd.dma_gather(
                            g1[:, t0:t0 + nt, :], src_ap,
                            gix[:, t0 * 8:(t0 + nt) * 8], nt * 128, nt * 128,
                            SROW, single_packet=False, queue_num=i,
                        )
                    eq_sb = p1pool.tile([128, ST, 128], bf16, tag="eqs1")
                    nc.sync.dma_start(
                        eq_sb[:],
                        p1_eq[:, toff * 128:(toff + ST) * 128].rearrange(
                            "l (t r) -> l t r", r=128
                        ),
                    )

                    # per-edge key-side s_src via on-chip select:
                    # eqT = transpose(eq); s_sel[lane] = eqT^T @ s_res[block]
                    tile_block = [0] * ST
                    for (b, lo0, tl, hi0, th) in m1.blocks[s]:
                        for t in range(lo0, lo0 + tl):
                            tile_block[t] = b
                        for t in range(hi0, hi0 + th):
                            tile_block[t] = b
                    zp = p1ps.tile([128, ST * HEADS], dt.float32, tag="zp")
                    ptr = eqt = None
                    for t in range(ST):
                        k = t % 4
                        if k == 0:
                            ptr = p1ps.tile([128, 512], bf16, tag="ptr")
                            eqt = p1pool.tile([128, 512], bf16, tag="eqt")
                        nc.tensor.transpose(
                            ptr[:, k * 128:(k + 1) * 128], eq_sb[:, t, :],
                            identb[:],
                        )
                        if k == 3 or t == ST - 1:
                            if (t // 4) % 2 == 0:
                                nc.scalar.copy(eqt[:, :(k + 1) * 128],
                                               ptr[:, :(k + 1) * 128])
                            else:
                                nc.vector.tensor_copy(eqt[:, :(k + 1) * 128],
                                                      ptr[:, :(k + 1) * 128])
                            for tt in range(t - k, t + 1):
                                kk = tt % 4
                                b = tile_block[tt]
                                nc.tensor.matmul(
                                    zp[:, tt * HEADS:(tt + 1) * HEADS],
                                    eqt[:, kk * 128:(kk + 1) * 128],
                                    s_res[:, b * H2:b * H2 + HEADS],
                                    start=True, stop=True,
                                )

                    # v = exp(leaky_relu(s_src + s_dst)) in bf16
                    z = p1pool.tile([128, ST * HEADS], dt.float32, tag="z1")
                    z3 = z[:].rearrange("p (t h) -> p t h", h=HEADS)
                    nc.vector.tensor_tensor(
                        z3, zp[:].rearrange("p (t h) -> p t h", h=HEADS),
                        g1[:, :, HEADS:H2], op=Alu.add
                    )
                    nc.vector.scalar_tensor_tensor(
                        z[:], z[:], 0.2, z[:], op0=Alu.mult, op1=Alu.max
                    )
                    v = p1pool.tile([128, ST * HEADS], bf16, tag="v1")
                    nc.scalar.activation(v[:], z[:], Act.Exp)

                    # per block: sumexp[128 nodes, 4] += eq_t^T @ v_t
                    for (b, lo0, tl, hi0, th) in m1.blocks[s]:
                        ps1 = p1ps.tile([128, HEADS], dt.float32, tag="ps1")
                        tiles = list(range(lo0, lo0 + tl)) + list(range(hi0, hi0 + th))
                        for i, t in enumerate(tiles):
                            nc.tensor.matmul(
                                ps1[:], eq_sb[:, t, :],
                                v[:, t * HEADS:(t + 1) * HEADS],
                                start=(i == 0), stop=(i == len(tiles) - 1),
                            )
                        se = p1pool.tile([128, HEADS], dt.float32, tag="se")
                        nc.vector.tensor_scalar_add(se[:], ps1[:], 1e-10)
                        recd = p1pool.tile([128, HEADS], dt.float32, tag="recd")
                        nc.vector.reciprocal(recd[:], se[:])
                        b0, b1 = b * 128, (b + 1) * 128
                        if b1 <= hcr:
                            nc.sync.dma_start(
                                G_ownA[b0:b1,
                                       F + H2:F + 2 * H2].bitcast(dt.float32),
                                recd[:],
                            )
                        elif b0 >= hcr:
                            nc.sync.dma_start(
                                G_ownB[b0 - hcr:b1 - hcr,
                                       F + H2:F + 2 * H2].bitcast(dt.float32),
                                recd[:],
                            )
                        else:
                            cut = hcr - b0
                            nc.sync.dma_start(
                                G_ownA[b0:hcr,
                                       F + H2:F + 2 * H2].bitcast(dt.float32),
                                recd[0:cut, :],
                            )
                            nc.sync.dma_start(
                                G_ownB[0:b1 - hcr,
                                       F + H2:F + 2 * H2].bitcast(dt.float32),
                                recd[cut:128, :],
                            )

            nc.gpsimd.collective_compute(
                "AllGather", Alu.bypass,
                ins=[G_ownA[:].opt()], outs=[G_fullA[:].opt()],
                replica_groups=[list(range(n_cores))],
            )
            dmy_g = cpool.tile([1, 8], bf16)
            nc.sync.dma_start(dmy_g[:], G_fullA[0:1, 0:8])
            nc.sync.dma_start(G_ownB[rpc - hcr - 1:rpc - hcr, GROW - 8:GROW], dmy_g[:])
            nc.gpsimd.collective_compute(
                "AllGather", Alu.bypass,
                ins=[G_ownB[:].opt()], outs=[G_fullB[:].opt()],
                replica_groups=[list(range(n_cores))],
            )

            # ---------------- phase 2: aggregate messages ----------------
            with (
                tc.tile_pool(name="p2", bufs=3) as p2pool,
                tc.tile_pool(name="p2m", bufs=3) as p2m,
                tc.tile_pool(name="p2ps", bufs=2, space="PSUM") as p2ps,
            ):
                for s in range(m2.nsup):
                    ST = m2.st[s]
                    STl = m2.st_lo[s]
                    STh = m2.st_hi[s]
                    toff = m2.sup_off[s]
                    co = toff * 8
                    gix = p2pool.tile([128, ST * 8], dt.int16, tag="gix2")
                    nc.sync.dma_start(gix[:], p2_gidx[:, co:co + ST * 8])

                    g = p2pool.tile([128, ST, GROW], bf16, tag="g")
                    La = STl // 2
                    Ha = STh // 2
                    calls = [
                        (0, La, G_fullA[:, :]),
                        (La, STl - La, G_fullA[:, :]),
                        (STl, Ha, G_fullB[:, :]),
                        (STl + Ha, STh - Ha, G_fullB[:, :]),
                    ]
                    for i, (t0, nt, src_ap) in enumerate(calls):
                        if nt == 0:
                            continue
                        nc.gpsimd.dma_gather(
                            g[:, t0:t0 + nt, :], src_ap,
                            gix[:, t0 * 8:(t0 + nt) * 8], nt * 128, nt * 128,
                            GROW, single_packet=False, queue_num=i,
                        )
                    eq_sb = p2pool.tile([128, ST, 128], bf16, tag="eqs2")
                    nc.sync.dma_start(
                        eq_sb[:],
                        p2_eq[:, toff * 128:(toff + ST) * 128].rearrange(
                            "l (t r) -> l t r", r=128
                        ),
                    )

                    # per-edge key-side s_dst via on-chip select (as phase 1)
                    tile_block = [0] * ST
                    for (b, lo0, tl, hi0, th) in m2.blocks[s]:
                        for t in range(lo0, lo0 + tl):
                            tile_block[t] = b
                        for t in range(hi0, hi0 + th):
                            tile_block[t] = b
                    zp = p2ps.tile([128, ST * HEADS], dt.float32, tag="zp2")
                    ptr = eqt = None
                    for t in range(ST):
                        k = t % 4
                        if k == 0:
                            ptr = p2ps.tile([128, 512], bf16, tag="ptr2")
                            eqt = p2pool.tile([128, 512], bf16, tag="eqt2")
                        nc.tensor.transpose(
                            ptr[:, k * 128:(k + 1) * 128], eq_sb[:, t, :],
                            identb[:],
                        )
                        if k == 3 or t == ST - 1:
                            if (t // 4) % 2 == 0:
                                nc.scalar.copy(eqt[:, :(k + 1) * 128],
                                               ptr[:, :(k + 1) * 128])
                            else:
                                nc.vector.tensor_copy(eqt[:, :(k + 1) * 128],
                                                      ptr[:, :(k + 1) * 128])
                            for tt in range(t - k, t + 1):
                                kk = tt % 4
                                b = tile_block[tt]
                                nc.tensor.matmul(
                                    zp[:, tt * HEADS:(tt + 1) * HEADS],
                                    eqt[:, kk * 128:(kk + 1) * 128],
                                    s_res[:, b * H2 + HEADS:(b + 1) * H2],
                                    start=True, stop=True,
                                )

                    # alpha = exp(leaky_relu(s_src + s_dst)) * recip  (f32)
                    al = p2pool.tile([128, ST * HEADS], dt.float32, tag="al")
                    al3 = al[:].rearrange("p (t h) -> p t h", h=HEADS)
                    nc.vector.tensor_tensor(
                        al3,
                        g[:, :, F:F + H2].bitcast(dt.float32),
                        zp[:].rearrange("p (t h) -> p t h", h=HEADS),
                        op=Alu.add,
                    )
                    nc.vector.scalar_tensor_tensor(
                        al[:], al[:], 0.2, al[:], op0=Alu.mult, op1=Alu.max
                    )
                    nc.scalar.activation(al[:], al[:], Act.Exp)
                    nc.vector.tensor_tensor(
                        al3, al3,
                        g[:, :, F + H2:F + 2 * H2].bitcast(dt.float32),
                        op=Alu.mult,
                    )

                    for (b, lo0, tl, hi0, th) in m2.blocks[s]:
                        ps2 = p2ps.tile([128, F], dt.float32, tag="ps2")
                        tiles = list(range(lo0, lo0 + tl)) + list(range(hi0, hi0 + th))
                        for i, t in enumerate(tiles):
                            alpha_b = al[:, t * HEADS:(t + 1) * HEADS].unsqueeze(
                                2
                            ).broadcast_to([128, HEADS, OUT_DIM])
                            msg = p2m.tile([128, F], bf16, tag="msg")
                            nc.vector.tensor_tensor(
                                msg[:].rearrange("p (h d) -> p h d", h=HEADS),
                                g[:, t, 0:F].rearrange("p (h d) -> p h d", h=HEADS),
                                alpha_b,
                                op=Alu.mult,
                            )
                            nc.tensor.matmul(
                                ps2[:], eq_sb[:, t, :], msg[:],
                                start=(i == 0), stop=(i == len(tiles) - 1),
                            )
                        osb = p2m.tile([128, F], dt.float32, tag="osb")
                        nc.vector.tensor_tensor(osb[:], ps2[:], bias_bc[:], op=Alu.add)
                        nc.sync.dma_start(out[b * 128:(b + 1) * 128, :], osb[:])

    nc.compile()
    return nc


def _gat_forward(x, edges, W, a, bias, n_nodes, n_cores, run_opts=None):
    npad = _ceil_div(n_nodes, n_cores * 128) * n_cores * 128
    rpc = npad // n_cores
    nblk = rpc // 128

    src = edges[:, 0].astype(np.int64)
    dst = edges[:, 1].astype(np.int64)
    m1 = PhaseMeta(src, dst, n_cores, nblk, SUP1)
    m2 = PhaseMeta(dst, src, n_cores, nblk, SUP2)

    nc = _build_bass_program(npad, rpc, nblk, m1, m2, n_cores)

    import ml_dtypes

    x_pad = np.zeros((npad, IN_DIM), np.float32)
    x_pad[:n_nodes] = x
    xT = np.ascontiguousarray(x_pad.T.astype(ml_dtypes.bfloat16))

    in_maps = []
    for c in range(n_cores):
        in_maps.append({
            "xT": np.ascontiguousarray(xT[:, c * rpc:(c + 1) * rpc]),
            "W": np.ascontiguousarray(W.astype(np.float32)),
            "a": np.ascontiguousarray(a.astype(np.float32)),
            "bias": np.ascontiguousarray(bias.astype(np.float32).reshape(1, F)),
            "p1_gidx": m1.gidx_c[c], "p1_eq": m1.eq_c[c],
            "p2_gidx": m2.gidx_c[c], "p2_eq": m2.eq_c[c],
        })

    from concourse.bass_utils import run_bass_kernel_spmd

    res = run_bass_kernel_spmd(
        nc, in_maps, core_ids=list(range(n_cores)), **(run_opts or {})
    )
    out = np.concatenate([r["out"] for r in res.results], axis=0)
    return out[:n_nodes], res


def kernel(x, edges, W, a, bias):
    x = np.asarray(x, np.float32)
    edges = np.asarray(edges)
    W = np.asarray(W, np.float32)
    a = np.asarray(a, np.float32)
    bias = np.asarray(bias, np.float32)
    out, _ = _gat_forward(x, edges, W, a, bias, N_NODES, N_CORES)
    return out
